# revision 57
# baseline (speedup 1.0000x reference)
"""Causal self-attention kernel for 8 TRN2 NeuronCores.

Sharding: 8 cores = 4 batches x 2 head-groups (8 heads / 512 channels each).
Each core computes q/k/v projections for its head half, causal attention for
its 8 heads, and a partial c_proj contracted over its 512 channels. The host
sums the two partials per batch and adds the c_proj bias.

QK^T runs in fp8e4m3 with DoubleRow packing (0.5 cycles/row), and the K
projection itself also runs in fp8 DoubleRow (x and 64*Wk quantized to
fp8e4m3 on the host; the 1/64 is folded into the softmax exp scale), which
quarters its matmul cost. The Q projection is half-fp8 (cin [0,512) in fp8
DoubleRow at x32 scale, the rest bf16); V and c_proj stay bf16 for precision
(measured rel err 1.85e-2 against the fp32 reference, budget 2e-2). Layouts:
  qt8/kt8 [128, hg, 2, T]  fp8 DR layout: partition 32*(h%4)+i, hg=h//4,
                           plane j holds d=32j+i of head h
  xt_sb  [128, 8, T]       bf16 x^T for the projections
  v1_sb  [128, T//128, 65, 8]  V interleaved + ones column (softmax denom)

Scores are computed transposed (S^T[k, q]); QK writes k-tile PAIRS into one
2-bank PSUM tile so a single ScalarE exp covers both tiles (halves ACT
instruction overhead). Probabilities are masked on VectorE (GPSIMD's ~350ns
per mask sat directly in the exp->P@V critical chain; DVE does it in ~150).

P@V is FLIPPED: the probability tile P^T[k, q-tile] is the stationary operand
and V[k, 65] the moving one, so each matmul streams only 65 columns while
using all 128 weight columns (2x the array efficiency of V-stationary).  The
output lands q-major ([q, d] + denominator column), which makes the softmax
denominator a per-partition scalar: normalization is one VectorE reciprocal +
one multiply (no DRAM-bounce broadcast).  Normalized [q, 2*64] head-pair
tiles are transposed back to [d, q] with PE transpose-mode matmuls against an
identity, then evacuated to yt_sb for the (unchanged) c_proj.
"""

import numpy as np
import ml_dtypes
from contextlib import ExitStack

import concourse.bass as bass
import concourse.tile as tile
from concourse import bacc, mybir
from concourse.bass_utils import run_bass_kernel_spmd

BF16 = mybir.dt.bfloat16
F32 = mybir.dt.float32
F8 = mybir.dt.float8e4
NP8 = ml_dtypes.float8_e4m3
DR = mybir.MatmulPerfMode.DoubleRow

N_EMBD = 1024
N_HEAD = 16
B = 4
T_FULL = 2048
HD = 64           # head dim
HPC = 8           # heads per core
CH = HPC * HD     # channels per core = 512
N_CORES = 8
SCALE = 1.0 / 8.0  # 1/sqrt(HD)
KSC = 64.0        # host pre-scale on Wk/bk so fp8 weights stay normal-range
QSC = 32.0        # host pre-scale on Wq/bq (half-fp8 Q projection)

P = 128           # partitions
QC = 512          # q-chunk (matmul free dim)


def build_nc(T=T_FULL):
    """Build the per-core Bass module (same program on every core)."""
    n_tt = T // P          # 128-row tiles along T
    n_qc = T // QC         # 512-wide chunks along T
    n_ci = N_EMBD // P     # bf16 contraction tiles over the full embed dim
    n_c2 = N_EMBD // 256   # fp8 DoubleRow contraction tiles (256 cin each)
    n_dt = CH // P         # d-tiles of this core's 512 channels (= head pairs)
    n_oc = N_EMBD // QC

    nc = bacc.Bacc("TRN2", target_bir_lowering=False, debug=False)

    # all bulk inputs arrive pre-arranged on the host into [partition, ...]
    # layouts so every DMA is contiguous per partition (fewer descriptors,
    # faster transfers than strided rearranges)
    n_cq = T // QC
    xt = nc.dram_tensor("xt", [P, n_cq, N_EMBD // P, QC], BF16,
                        kind="ExternalInput").ap()
    # Q projection is half-fp8: cin [0, 512) contracts in fp8 DoubleRow
    # (weights pre-scaled by QSC), cin [512, 1024) in bf16 (also x QSC so
    # both halves accumulate on the same scale); 1/QSC folds into the exp
    wq8 = nc.dram_tensor("wq8", [P, N_EMBD // 512, 2, CH], F8,
                         kind="ExternalInput").ap()
    wq = nc.dram_tensor("wq", [P, CH // P, N_EMBD // (2 * P), P], BF16,
                        kind="ExternalInput").ap()
    wk8 = nc.dram_tensor("wk8", [P, N_EMBD // 256, 2, CH], F8,
                         kind="ExternalInput").ap()
    x8 = nc.dram_tensor("x8", [P, n_cq, N_EMBD // P, QC], F8,
                        kind="ExternalInput").ap()
    wv = nc.dram_tensor("wv", [P, N_EMBD // P, CH], BF16,
                        kind="ExternalInput").ap()
    wc = nc.dram_tensor("wc", [P, CH // P, N_EMBD], BF16,
                        kind="ExternalInput").ap()
    bq = nc.dram_tensor("bq", [P, n_dt], F32, kind="ExternalInput").ap()
    bk = nc.dram_tensor("bk", [P, n_dt], F32, kind="ExternalInput").ap()
    vb1 = nc.dram_tensor("vb1", [P, CH + HPC], F32, kind="ExternalInput").ap()
    tri = nc.dram_tensor("tri", [P, P], BF16, kind="ExternalInput").ap()
    eye = nc.dram_tensor("eye", [P, P], BF16, kind="ExternalInput").ap()
    out = nc.dram_tensor("out", [T, N_EMBD], F32, kind="ExternalOutput").ap()

    with tile.TileContext(nc) as tc, ExitStack() as ctx:
        singles = ctx.enter_context(tc.tile_pool(name="singles", bufs=1))
        mm_ps = ctx.enter_context(tc.tile_pool(name="mm_ps", bufs=2, space="PSUM"))
        qk_ps_pool = ctx.enter_context(tc.tile_pool(name="qk_ps", bufs=2, space="PSUM"))
        av_ps_pool = ctx.enter_context(tc.tile_pool(name="av_ps", bufs=2, space="PSUM"))
        pt_pool = ctx.enter_context(tc.tile_pool(name="pt", bufs=6))
        tmp8 = ctx.enter_context(tc.tile_pool(name="tmp8", bufs=4))
        small = ctx.enter_context(tc.tile_pool(name="small", bufs=4))
        yqd_pool = ctx.enter_context(tc.tile_pool(name="yqd", bufs=2))
        ost = ctx.enter_context(tc.tile_pool(name="ost", bufs=4))

        # ---- resident tensors. DMAs are split and ordered by first use so
        # compute starts as soon as the first slices land (HWDGE dispatches
        # one DMA per ~625ns; transfers then run concurrently) ----
        xt_sb = singles.tile([P, n_cq, n_ci, QC], BF16)
        wq8_sb = singles.tile([P, n_c2 // 2, 2, CH], F8)
        wq_sb = singles.tile([P, n_dt, n_ci // 2, P], BF16)
        wk8_sb = singles.tile([P, n_c2, 2, CH], F8)
        x8_sb = singles.tile([P, n_cq, n_ci, QC], F8)
        wv_sb = singles.tile([P, n_ci, CH], BF16)
        bq_sb = singles.tile([P, n_dt], F32)
        bk_sb = singles.tile([P, n_dt], F32)
        vb1_sb = singles.tile([P, CH + HPC], F32)
        tri_sb = singles.tile([P, P], BF16)
        eye_sb = singles.tile([P, P], BF16)
        wc_sb = singles.tile([P, n_dt, N_EMBD], BF16)

        hci = n_ci // 2
        # critical path to the first QK pair first: k inputs, then q, with
        # the small bias tensors after the bulk they follow in compute order
        nc.sync.dma_start(wk8_sb, wk8)
        nc.sync.dma_start(wq8_sb, wq8)
        nc.sync.dma_start(x8_sb[:, 0], x8[:, 0])
        nc.sync.dma_start(xt_sb[:, 0, 0:hci, :], xt[:, 0, 0:hci, :])
        nc.sync.dma_start(xt_sb[:, 0, hci:, :], xt[:, 0, hci:, :])
        nc.sync.dma_start(wq_sb[:, 0], wq[:, 0])
        nc.sync.dma_start(bk_sb, bk)
        nc.sync.dma_start(bq_sb, bq)
        nc.sync.dma_start(wv_sb[:, 0:hci], wv[:, 0:hci])
        nc.sync.dma_start(wv_sb[:, hci:], wv[:, hci:])
        nc.sync.dma_start(vb1_sb, vb1)
        nc.sync.dma_start(tri_sb, tri)
        nc.sync.dma_start(eye_sb, eye)
        for pr in range(1, n_dt):
            nc.sync.dma_start(wq_sb[:, pr], wq[:, pr])
        # bulk x slices for later chunks and wc are emitted inside the chunk
        # loop (one chunk ahead of first use) so their SP-queue FIFO slots
        # come after the latency-critical q/k rearranges of earlier chunks
        def emit_late_inputs(c):
            """DMA x chunk c (bf16 + fp8); c >= 1."""
            nc.sync.dma_start(xt_sb[:, c, 0:hci, :], xt[:, c, 0:hci, :])
            nc.sync.dma_start(xt_sb[:, c, hci:, :], xt[:, c, hci:, :])
            nc.sync.dma_start(x8_sb[:, c], x8[:, c])

        # fp8 q/k on partitions 0-63: [32*hh+ii, pr, j, t]; head h=2pr+hh
        # owns partitions [32hh, 32hh+32), plane j holds d=32j+ii
        qt8_sb = singles.tile([64, n_dt, 2, T], F8)
        kt8_sb = singles.tile([64, n_dt, 2, T], F8)
        v1_sb = singles.tile([P, n_tt, HD + 1, HPC], BF16)
        yt_sb = singles.tile([P, n_dt, T], BF16)   # attention out (normalized)

        # preload the ScalarE exp table set during the input-DMA window so
        # the first real exp doesn't pay the ACT_TABLE_LOAD stall
        warm1 = small.tile([1, 1], F32, tag="warm1")
        nc.vector.memset(warm1, 0.0)
        nc.scalar.activation(warm1, warm1, mybir.ActivationFunctionType.Exp)

        # spin the PE p-state up to full clock during the input-DMA window
        # with garbage matmuls on a zeroed tile (results never read), so the
        # first real projection matmuls run at 2.4 GHz instead of ramping
        warm_sb = small.tile([P, QC], BF16, tag="warm")
        nc.vector.memset(warm_sb, 0.0)
        wps0 = qk_ps_pool.tile([P, 2, QC], F32, tag="qk", name="warm_start")
        for i in range(10):
            n = P if i < 4 else QC
            nc.tensor.matmul(wps0[:, 0, 0:n], lhsT=warm_sb[:, 0:P],
                             rhs=warm_sb[:, 0:n], start=True, stop=True)

        # ones column of v1 (written once, before any V tile is consumed)
        nc.vector.tensor_copy(
            out=v1_sb[:, :, HD, :],
            in_=vb1_sb[:, None, CH:].to_broadcast((P, n_tt, HPC)),
        )

        prologue = [True]

        # ---- work units (generators yielding after each matmul with its
        # approximate PE-cycle cost, so filler work can be interleaved at
        # sub-unit granularity between attention pairs) ----
        def emit_v(tt, tag="mm"):
            ps = mm_ps.tile([P, CH], F32, tag=tag, name=f"v_{tt}")
            for ci in range(n_ci):
                nc.tensor.matmul(
                    ps,
                    lhsT=xt_sb[:, tt // 4, ci, (tt % 4) * P:(tt % 4 + 1) * P],
                    rhs=wv_sb[:, ci, :],
                    start=(ci == 0), stop=(ci == n_ci - 1),
                )
                yield 512
            nc.vector.tensor_add(
                out=v1_sb[:, tt, 0:HD, :],
                in0=ps.rearrange("p (j h) -> p j h", h=HPC),
                in1=vb1_sb[:, 0:CH].rearrange("p (j h) -> p j h", h=HPC),
            )

        def finish_proj(ps, b_sb, dst, pr, tcn, name):
            """bias-add casts the projection PSUM to fp8, then two DMAs move
            the j-plane partition blocks into the qt8/kt8 DoubleRow layout.
            The DMAs ride the SP HWDGE queue: everything else there is
            latency-tolerant, and a DMACopy occupies its queue's sequencer
            until the transfer completes, so placing these on the Activation
            queue would starve the exp dispatch."""
            t8 = tmp8.tile([P, QC], F8, tag="tmp8")
            nc.vector.tensor_add(
                out=t8, in0=ps, in1=b_sb[:, pr, None].to_broadcast((P, QC)))
            # prologue rearranges ride the (still-idle) Activation queue to
            # skip the input backlog on SP; steady-state ones go to SP where
            # nothing is latency-critical
            q = nc.scalar if prologue[0] else nc.sync
            for j in range(2):
                q.dma_start(
                    out=dst[0:64, pr, j, tcn * QC:(tcn + 1) * QC],
                    in_=t8[64 * j:64 * j + 64, :],
                )

        def emit_proj_q(pr, tcn):
            """half-fp8 projection of QSC*q: cin [0, 512) in fp8 DoubleRow,
            cin [512, 1024) in bf16, accumulating into one PSUM group."""
            ps = mm_ps.tile([P, QC], F32, tag="mm", name=f"pjq_{pr}_{tcn}")
            for c2 in range(n_c2 // 2):
                nc.tensor.matmul(
                    ps,
                    lhsT=wq8_sb[:, c2, :, pr * P:(pr + 1) * P],
                    rhs=x8_sb[:, tcn, 2 * c2:2 * c2 + 2, :],
                    start=(c2 == 0), stop=False,
                    perf_mode=DR,
                )
                yield 256
            for ci in range(n_ci // 2):
                nc.tensor.matmul(
                    ps,
                    lhsT=wq_sb[:, pr, ci, :],
                    rhs=xt_sb[:, tcn, n_ci // 2 + ci, :],
                    start=False, stop=(ci == n_ci // 2 - 1),
                )
                yield 512
            finish_proj(ps, bq_sb, qt8_sb, pr, tcn, "q")

        def emit_proj_k(pr, tcn):
            """fp8 DoubleRow projection of 64*k (weights pre-scaled by 64 on
            the host; the 1/64 is folded into the softmax exp scale)."""
            ps = mm_ps.tile([P, QC], F32, tag="mm", name=f"pjk_{pr}_{tcn}")
            for c2 in range(n_c2):
                nc.tensor.matmul(
                    ps,
                    lhsT=wk8_sb[:, c2, :, pr * P:(pr + 1) * P],
                    rhs=x8_sb[:, tcn, 2 * c2:2 * c2 + 2, :],
                    start=(c2 == 0), stop=(c2 == n_c2 - 1),
                    perf_mode=DR,
                )
                yield 256
            finish_proj(ps, bk_sb, kt8_sb, pr, tcn, "k")

        def emit_cproj(tt):
            # both oc halves of the row-tile: 2 matmul groups, one out DMA
            st = ost.tile([P, n_oc, QC], F32, tag="ost", name=f"ost_{tt}")
            for oc in range(n_oc):
                ps = mm_ps.tile([P, QC], F32, tag="mm")
                for pp in range(n_dt):
                    nc.tensor.matmul(
                        ps,
                        lhsT=yt_sb[:, pp, tt * P:(tt + 1) * P],
                        rhs=wc_sb[:, pp, oc * QC:(oc + 1) * QC],
                        start=(pp == 0), stop=(pp == n_dt - 1),
                    )
                    yield 512
                nc.vector.tensor_copy(out=st[:, oc, :], in_=ps)
                # per-oc store so the last tile's first half is in flight
                # while its second half is still in the array
                nc.sync.dma_start(
                    out=out[tt * P:(tt + 1) * P, oc * QC:(oc + 1) * QC],
                    in_=st[:, oc, :],
                )

        def run_gen(g):
            for _ in g:
                pass

        # ---- prologue: head-pair-0 projections first (their rearrange
        # DMAs gate the first QK pair; k first since its DMAs finish last),
        # then the chunk-0 V tiles ----
        run_gen(emit_proj_k(0, 0))
        run_gen(emit_proj_q(0, 0))
        for tt in range(QC // P):
            run_gen(emit_v(tt))

        prologue[0] = False

        # ---- fused pipeline over q-chunks ----
        order = list(range(n_qc))
        nqt = QC // P  # q-tiles per chunk (4)

        # global filler queue: projection/c_proj units appended per chunk,
        # consumed by cycle allowance between attention pairs, with
        # per-(chunk, head-pair) markers force-completed one iteration ahead
        # of first use so rearrange-DMA latency is always covered
        gfill = []
        marks = {}
        pump_state = [0.0, 0, None]  # [allowance cycles, next idx, gen]

        def make_gen(f):
            if f[0] == "v":
                return emit_v(f[1])
            elif f[0] == "q":
                return emit_proj_q(f[1], f[2])
            elif f[0] == "k":
                return emit_proj_k(f[1], f[2])
            else:
                return emit_cproj(f[1])

        def pump(extra=0.0):
            pump_state[0] += extra
            while pump_state[0] > 0:
                if pump_state[2] is None:
                    if pump_state[1] >= len(gfill):
                        return
                    pump_state[2] = make_gen(gfill[pump_state[1]])
                    pump_state[1] += 1
                try:
                    while pump_state[0] > 0:
                        pump_state[0] -= next(pump_state[2])
                except StopIteration:
                    pump_state[2] = None

        def pump_until(idx):
            """Force-complete fillers [0, idx) regardless of allowance."""
            while pump_state[1] < idx or (pump_state[1] == idx
                                          and pump_state[2] is not None):
                if pump_state[2] is None:
                    pump_state[2] = make_gen(gfill[pump_state[1]])
                    pump_state[1] += 1
                for _ in pump_state[2]:
                    pass
                pump_state[2] = None

        unit_cost = {"v": 4096, "q": 2560, "k": 1024, "c": 4096}

        vmarks = {}

        def append_batch(c):
            """Queue chunk c's q/k projections (V tiles are queued at the
            start of chunk c's own phase: the attention only touches chunk
            c's V tiles from pair 4c on, and deferring them keeps PE fed
            through the later, exp-bound phases)."""
            for pr in range(n_dt):
                gfill.append(("q", pr, c))
                gfill.append(("k", pr, c))
                marks[(c, pr)] = len(gfill)

        # chunk 0's remaining head-pair projections are the earliest fillers
        for pr in range(1, n_dt):
            gfill.append(("q", pr, 0))
            gfill.append(("k", pr, 0))
            marks[(0, pr)] = len(gfill)

        # deferred transpose+evac: (pr, q0, yqd) from the previous head-pair
        # iteration, flushed once the next iteration's second QK pair is in
        # flight so the PE never waits on the normalize latency chain
        pending_tr = [None]

        def flush_tr():
            if pending_tr[0] is None:
                return
            fpr, fq0, fyqd = pending_tr[0]
            pending_tr[0] = None
            # transpose normalized [q, (hi d)] tiles back to [(hi d), q]
            tr = av_ps_pool.tile([P, nqt, P], BF16, tag="av")
            for qt in range(nqt):
                nc.tensor.matmul(
                    tr[:, qt, :],
                    lhsT=fyqd[:, qt].rearrange("p a b -> p (a b)"),
                    rhs=eye_sb,
                    start=(qt == 0), stop=(qt == nqt - 1),
                    is_transpose=True,
                )
            nc.vector.tensor_copy(
                out=yt_sb[:, fpr, fq0:fq0 + QC],
                in_=tr.rearrange("p a q -> p (a q)"),
            )

        # deferred iteration tail: the last P@V drains and the normalize of
        # one (pr, hi) iteration run after the NEXT iteration's first QK
        # pair + exp are queued, so the exp stream never starves at the
        # head-pair boundary
        pending_fin = [None]

        def flush_fin():
            if pending_fin[0] is None:
                return
            fin = pending_fin[0]
            pending_fin[0] = None
            fin()

        for s, qcn in enumerate(order):
            q0 = qcn * QC
            nkt = (q0 + QC) // P  # causal: k-tiles 0..nkt-1
            last = s == n_qc - 1
            if qcn >= 1:
                # this chunk's own V tiles, first consumed at pair 4*qcn
                for tt in range(qcn * nqt, (qcn + 1) * nqt):
                    gfill.append(("v", tt))
                vmarks[qcn] = len(gfill)
            if s + 1 < n_qc:
                emit_late_inputs(order[s + 1])
                append_batch(order[s + 1])
            if s == max(0, n_qc - 2):
                nc.sync.dma_start(wc_sb, wc)
            if s == n_qc - 2 and n_qc > 2:
                # top up the second-to-last (exp-bound) phase with a couple
                # of finished c_proj tiles
                gfill.append(("c", 0))
                gfill.append(("c", 1))
            if last:
                # deferrable c_proj for all earlier chunks
                for done in order[:-1]:
                    for tt in range(done * nqt, (done + 1) * nqt):
                        if n_qc > 2 and tt in (0, 1):
                            continue
                        gfill.append(("c", tt))

            remaining = sum(unit_cost[f[0]] for f in gfill[pump_state[1]:])
            n_slots = n_dt * nkt
            per_slot = remaining / n_slots
            if last:
                per_slot *= 0.9  # hold a little filler back for the chunk-end drain

            for pr in range(n_dt):
                # force-complete the NEXT iteration's q/k fillers now, one
                # iteration of lead over their first QK pair
                if pr + 1 < n_dt:
                    la = marks.get((qcn, pr + 1))
                elif s + 1 < n_qc:
                    la = marks.get((order[s + 1], 0))
                else:
                    la = None
                if la is not None:
                    pump_until(la)
                # normalized [q, qt, hi, d] head-pair output (qt-major so a
                # per-qt slice is one contiguous free dim for the transpose)
                yqd = yqd_pool.tile([P, nqt, 2, HD], BF16, tag="yqd")
                for hi in range(2):
                    hp = 32 * hi
                    # flipped P@V accumulator: [q, qt, d + denominator]
                    av = av_ps_pool.tile([P, nqt, HD + 1], F32, tag="av")
                    pend = []  # delayed P@V queue: (kt2, pt)

                    def drain_pair(av=av, pend=pend, pr=pr, hi=hi, nkt=nkt,
                                   q0=q0):
                        pkt2, ppt = pend.pop(0)
                        for u in range(2):
                            kt = pkt2 + u
                            for qt in range(max(0, kt - q0 // P), nqt):
                                nc.tensor.matmul(
                                    av[:, qt, :],
                                    lhsT=ppt[:, u, qt * P:(qt + 1) * P],
                                    rhs=v1_sb[:, kt, :, 2 * pr + hi],
                                    start=(kt == 0 and qt == 0),
                                    stop=(kt == nkt - 1 and qt == nqt - 1),
                                )

                    for kt2 in range(0, nkt, 2):
                        if qcn >= 1 and kt2 == 4 * qcn:
                            # safety: this chunk's V tiles before their first
                            # P@V drain could need them
                            pump_until(vmarks[qcn])
                        rel0 = kt2 - (q0 // P)
                        relq0 = rel0 * P if rel0 > 0 else 0
                        qkp = qk_ps_pool.tile([P, 2, QC], F32, tag="qk")
                        for u in range(2):
                            # both tiles of the pair span [relq0:] so the
                            # paired exp reads fully-written PSUM; the
                            # triangle mask zeroes the invalid bands
                            kt = kt2 + u
                            k0 = kt * P
                            nc.tensor.matmul(
                                qkp[:, u, relq0:],
                                lhsT=kt8_sb[hp:hp + 32, pr, :, k0:k0 + P],
                                rhs=qt8_sb[hp:hp + 32, pr, :, q0 + relq0:q0 + QC],
                                start=True, stop=True,
                                perf_mode=DR,
                                tile_position=(hp, 0),
                            )
                        pt = pt_pool.tile([P, 2, QC], BF16, tag="pt")
                        nc.scalar.activation(
                            pt[:, :, relq0:], qkp[:, :, relq0:],
                            mybir.ActivationFunctionType.Exp,
                            scale=SCALE / (KSC * QSC),
                        )
                        for u in range(2):
                            rel = kt2 + u - (q0 // P)
                            if rel >= 0:
                                # mask only the 128-wide diagonal triangle
                                # band (everything else is already valid);
                                # runs on GPSIMD to keep DVE/ACT free
                                rq = rel * P
                                nc.vector.tensor_mul(
                                    pt[:, u, rq:rq + P], pt[:, u, rq:rq + P],
                                    tri_sb,
                                )
                        pend.append((kt2, pt))
                        if kt2 == 0:
                            # previous iteration's tail drains + normalize,
                            # now that this iteration's first exp is queued
                            flush_fin()
                        elif kt2 == 2 and hi == 0:
                            # previous head pair's transposes, one pair after
                            # its hi=1 normalize was emitted by flush_fin
                            flush_tr()
                        if len(pend) > 2:
                            drain_pair()
                        # interleave filler work to keep TensorE fed
                        pump(per_slot)

                    def fin(av=av, pend=pend, yqd=yqd, hi=hi,
                            drain_pair=drain_pair):
                        while pend:
                            drain_pair()
                        # normalize: denominator is per-partition scalar here
                        r_sb = small.tile([P, nqt], F32, tag="recip")
                        nc.vector.reciprocal(out=r_sb, in_=av[:, :, HD])
                        nc.vector.tensor_mul(
                            out=yqd[:, :, hi, :],
                            in0=av[:, :, 0:HD],
                            in1=r_sb[:, :, None].to_broadcast((P, nqt, HD)),
                        )
                    pending_fin[0] = fin

                # hand the transpose+evac to the next iteration
                pending_tr[0] = (pr, q0, yqd)

        # leftover fillers, then the deferred tails of the last head pair
        pump(float("inf"))
        flush_fin()
        for i in range(4):
            wps = qk_ps_pool.tile([P, 2, QC], F32, tag="qk", name=f"warm_{i}")
            nc.tensor.matmul(
                wps[:, 0, :], lhsT=wc_sb[:, 0, 0:P], rhs=wc_sb[:, 0, 0:QC],
                start=True, stop=True,
            )
        flush_tr()
        for i in range(4, 8):
            wps = qk_ps_pool.tile([P, 2, QC], F32, tag="qk", name=f"warm_{i}")
            nc.tensor.matmul(
                wps[:, 0, :], lhsT=wc_sb[:, 0, 0:P], rhs=wc_sb[:, 0, 0:QC],
                start=True, stop=True,
            )

        # epilogue: c_proj for the final-stage chunk
        for tt in range(order[-1] * nqt, (order[-1] + 1) * nqt):
            run_gen(emit_cproj(tt))

    nc.compile()
    return nc


def make_in_maps(x, Wq, bq, Wk, bk, Wv, bv, T=T_FULL):
    """Host-side sharding + layout prep. Returns per-core input dicts."""
    bf = ml_dtypes.bfloat16
    x = np.asarray(x, dtype=np.float32)
    n_dt = CH // P
    n_c2 = N_EMBD // 256

    # triangle mask for the 128-wide diagonal band: tri[m, n] = (m <= n)
    tri = (np.arange(P)[:, None] <= np.arange(P)[None, :]).astype(bf)
    eye = np.eye(P, dtype=bf)

    # head-interleave permutation for Wv columns: new col j*HPC+h = old h*HD+j
    j = np.arange(HD)[:, None]
    h = np.arange(HPC)[None, :]
    perm = (h * HD + j).reshape(-1)  # new[j*HPC+h] <- old[h*HD+j]

    # q/k weight-column order so the j-plane blocks are partition-contiguous
    # in PSUM: new col m = pr*128 + j*64 + hh*32 + ii
    #   <- channel (2pr+hh)*64 + 32j + ii
    m = np.arange(CH)
    pr_i, r = m // 128, m % 128
    jj, hh_i, ii = r // 64, (r % 64) // 32, r % 32
    perm_qk = (2 * pr_i + hh_i) * HD + 32 * jj + ii

    def qk_bias(bvec):
        """[128, n_dt]: partition j*64+32hh+ii -> channel (2pr+hh)*64+32j+ii"""
        return np.ascontiguousarray(
            bvec[perm_qk].reshape(n_dt, P).T.astype(np.float32))

    wqT = np.ascontiguousarray(np.asarray(Wq, np.float32).T)  # [cin, dout]
    wkT = np.ascontiguousarray(np.asarray(Wk, np.float32).T)
    wvT = np.ascontiguousarray(Wv.T).astype(bf)

    in_maps = []
    for core in range(N_CORES):
        b = core // 2
        hh = core % 2
        cs = slice(hh * CH, (hh + 1) * CH)
        xtb = np.ascontiguousarray(x[b, :T].T)  # [N_EMBD, T] f32
        xt_bf = xtb.astype(bf)
        wq_perm = wqT[:, cs][:, perm_qk] * QSC
        wq8_arr = np.ascontiguousarray(
            wq_perm[:512].astype(NP8).reshape(2, 2, P, CH).transpose(2, 0, 1, 3))
        wq_arr = np.ascontiguousarray(wq_perm[512:]).astype(bf)
        # K weights pre-scaled by KSC and packed into the DoubleRow layout
        # wk8[p, c2, j, col] = KSC * WkT[256*c2 + 128*j + p, col']
        wk_s = (wkT[:, cs][:, perm_qk] * KSC).astype(NP8)
        wk8 = np.ascontiguousarray(
            wk_s.reshape(n_c2, 2, P, CH).transpose(2, 0, 1, 3))

        bq_arr = qk_bias(np.asarray(bq[cs], np.float32)) * QSC
        bk_arr = qk_bias(np.asarray(bk[cs], np.float32)) * KSC
        bv_half = np.asarray(bv[cs], np.float32)
        vb1 = np.concatenate([bv_half[perm], np.ones(HPC, np.float32)])
        vb1 = np.broadcast_to(vb1, (P, CH + HPC)).copy()

        n_ci = N_EMBD // P
        n_cq = T // QC

        def pc_x(a):
            """[N_EMBD, T] -> [P, chunk, ci, QC] partition-contiguous."""
            return np.ascontiguousarray(
                a.reshape(n_ci, P, n_cq, QC).transpose(1, 2, 0, 3))

        im = {
            "xt": pc_x(xt_bf),
            "wq8": wq8_arr,
            "wq": np.ascontiguousarray(
                wq_arr.reshape(n_ci // 2, P, n_dt, P).transpose(1, 2, 0, 3)),
            "wk8": wk8,
            "x8": pc_x(xtb.astype(NP8)),
            "wv": np.ascontiguousarray(
                wvT[:, cs][:, perm].reshape(n_ci, P, CH).transpose(1, 0, 2)),
            "wc": None,  # filled by caller (needs Wc)
            "bq": bq_arr,
            "bk": bk_arr,
            "vb1": vb1,
            "tri": tri,
            "eye": eye,
        }
        in_maps.append(im)
    return in_maps


_NC_CACHE = {}


def kernel(x, Wq, bq, Wk, bk, Wv, bv, Wc, bc):
    x = np.asarray(x, dtype=np.float32)
    T = x.shape[1]
    key = T
    if key not in _NC_CACHE:
        _NC_CACHE[key] = build_nc(T=T)
    nc = _NC_CACHE[key]

    in_maps = make_in_maps(x, Wq, bq, Wk, bk, Wv, bv, T=T)
    wcT = np.ascontiguousarray(np.asarray(Wc, np.float32).T).astype(
        ml_dtypes.bfloat16)  # [cin, cout]
    for core in range(N_CORES):
        hh = core % 2
        blk = wcT[hh * CH:(hh + 1) * CH, :]
        in_maps[core]["wc"] = np.ascontiguousarray(
            blk.reshape(CH // P, P, N_EMBD).transpose(1, 0, 2))

    res = run_bass_kernel_spmd(nc, in_maps, core_ids=list(range(N_CORES)))

    bc = np.asarray(bc, np.float32)
    out = np.empty((B, T, N_EMBD), np.float32)
    for b in range(B):
        out[b] = res.results[2 * b]["out"] + res.results[2 * b + 1]["out"] + bc
    return out

